# revision 12
# baseline (speedup 1.0000x reference)
"""CubeRecommender Trainium2 kernel (8-core SPMD).

Strategy:
  - Gallery (card) axis sharded 8 ways for the TransformCards MLP + similarity.
  - Batch axis sharded 8 ways for the EmbedCube set-encoder (gather + MLP +
    mean-pool); the tiny [B,E] cube encodings are AllGathered on-device.
  - Matmul inputs in fp16 (fp32 PSUM accumulation); l2norm / sigmoid in fp32.

Device program (identical on all 8 cores, per-core data differs):
  encoder:  dma_gather(transpose=True) pulls X^T tiles [128_D, 4, items] from
            the fp16 gallery; h = relu(X @ We1 + be1) via PE (X^T tiles as
            stationary operand); mean-pool via PE matmul with a 0/1
            block-diagonal mask; enc = pooled @ We2 + be2; AllGather enc.
  transform: shard^T via xbar DMA-transpose; th = relu(shard @ Wt1 + bt1);
            t2 = th @ Wt2 + bt2 (computed transposed: [E, cards]).
  sims:     sumsq per card via squared-tile x ones matmul; decoded^T tile =
            sigmoid(raw_sims * (rsqrt(sumsq)*T) - 0.5*T) with per-partition
            activation scale/bias.

Host glue: shard/layout prep, fp16 cast, and final unshard/transpose only.
"""

import numpy as np

NUM_CARDS = 30000
NPAD = 30720          # 8 * 3840, zero-padded gallery
SHARD = 3840          # cards per core (uniform SPMD shapes)
D = 512
H = 512
E = 256
B = 256
BP = 32               # batches per core
S = 360
NCORES = 8
ITEMS = BP * S        # 11520 gathered rows per core
ITILES = ITEMS // 128  # 90
GCHUNKS = 6
GCH = ITEMS // GCHUNKS  # 1920 indices per gather
CTILES = GCH // 128     # 15
NT = 480               # transform free-dim tile (3840 = 8*480)
NTT = SHARD // NT      # 8
CARD_TILES = SHARD // 128  # 30
EPS = 1e-12

_CACHE = {}


def build_program(parts="all"):
    import concourse.bass as bass
    import concourse.mybir as mybir
    import concourse.tile as tile
    from contextlib import ExitStack
    from concourse.masks import make_identity
    from concourse import bacc, library_config

    f16 = mybir.dt.float16
    f32 = mybir.dt.float32
    i16 = mybir.dt.int16
    AF = mybir.ActivationFunctionType
    ALU = mybir.AluOpType

    nc = bacc.Bacc("TRN2", target_bir_lowering=False, num_devices=NCORES)

    table = nc.dram_tensor("table", [NPAD, D], f16, kind="ExternalInput")
    shard = nc.dram_tensor("shard", [SHARD, D], f16, kind="ExternalInput")
    idx = nc.dram_tensor("idx", [128, ITEMS // 16], i16, kind="ExternalInput")
    maskd = nc.dram_tensor("maskd", [128, ITILES * BP], f16, kind="ExternalInput")
    we1 = nc.dram_tensor("we1", [128, 4, H], f16, kind="ExternalInput")
    we2 = nc.dram_tensor("we2", [128, 4, E], f16, kind="ExternalInput")
    wt1 = nc.dram_tensor("wt1", [128, 4, H], f16, kind="ExternalInput")
    wt2 = nc.dram_tensor("wt2", [128, 4, E], f16, kind="ExternalInput")
    be1 = nc.dram_tensor("be1", [1, H], f16, kind="ExternalInput")
    be2 = nc.dram_tensor("be2", [1, E], f16, kind="ExternalInput")
    bt1 = nc.dram_tensor("bt1", [128, 4], f32, kind="ExternalInput")
    bt2 = nc.dram_tensor("bt2", [128, 2], f32, kind="ExternalInput")
    temp = nc.dram_tensor("temp", [128, 1], f32, kind="ExternalInput")

    decT = nc.dram_tensor("decT", [SHARD, B], f32, kind="ExternalOutput")
    enco = nc.dram_tensor("enco", [BP, E], f32, kind="ExternalOutput")

    with ExitStack() as ctx:
        tc = ctx.enter_context(tile.TileContext(nc))
        singles = ctx.enter_context(tc.tile_pool(name="singles", bufs=1))
        dram = ctx.enter_context(tc.tile_pool(name="dram", bufs=1, space="DRAM"))
        gpool = ctx.enter_context(tc.tile_pool(name="gpool", bufs=2))
        hpool = ctx.enter_context(tc.tile_pool(name="hpool", bufs=3))
        spool = ctx.enter_context(tc.tile_pool(name="spool", bufs=2))
        outp = ctx.enter_context(tc.tile_pool(name="outp", bufs=3))
        mmps = ctx.enter_context(tc.tile_pool(name="mmps", bufs=4, space="PSUM"))
        smps = ctx.enter_context(tc.tile_pool(name="smps", bufs=1, space="PSUM"))
        sqps = ctx.enter_context(tc.tile_pool(name="sqps", bufs=1, space="PSUM"))
        tpps = ctx.enter_context(tc.tile_pool(name="tpps", bufs=1, space="PSUM"))

        # ---- constants / weights into SBUF ----
        we1_sb = singles.tile([128, 4, H], f16)
        nc.sync.dma_start(out=we1_sb[:], in_=we1[:])
        we2_sb = singles.tile([128, 4, E], f16)
        nc.sync.dma_start(out=we2_sb[:], in_=we2[:])
        wt1_sb = singles.tile([128, 4, H], f16)
        nc.sync.dma_start(out=wt1_sb[:], in_=wt1[:])
        wt2_sb = singles.tile([128, 4, E], f16)
        nc.sync.dma_start(out=wt2_sb[:], in_=wt2[:])
        be1_sb = singles.tile([1, H], f16)
        nc.sync.dma_start(out=be1_sb[:], in_=be1[:])
        be2_sb = singles.tile([1, E], f16)
        nc.sync.dma_start(out=be2_sb[:], in_=be2[:])
        bt1_sb = singles.tile([128, 4], f32)
        nc.sync.dma_start(out=bt1_sb[:], in_=bt1[:])
        bt2_sb = singles.tile([128, 2], f32)
        nc.sync.dma_start(out=bt2_sb[:], in_=bt2[:])
        temp_sb = singles.tile([128, 1], f32)
        nc.sync.dma_start(out=temp_sb[:], in_=temp[:])
        mask_sb = singles.tile([128, ITILES * BP], f16)
        nc.sync.dma_start(out=mask_sb[:], in_=maskd[:])
        idx_sb = singles.tile([128, ITEMS // 16], i16)
        nc.sync.dma_start(out=idx_sb[:], in_=idx[:])

        ones1_sb = singles.tile([1, 128], f16)
        nc.vector.memset(ones1_sb[:], 1.0)
        onesE_sb = singles.tile([128, 1], f16)
        nc.vector.memset(onesE_sb[:], 1.0)
        ident_sb = singles.tile([128, 128], f32)
        make_identity(nc, ident_sb[:])
        nc.gpsimd.load_library(library_config.mlp)

        # shard^T: [128_D, 4, SHARD] fp16 via xbar DMA-transpose
        shardT_sb = singles.tile([128, 4, SHARD], f16)
        for c in range(4 if parts in ("all", "trans") else 0):
            nc.sync.dma_start_transpose(
                out=shardT_sb[:, c, :], in_=shard[:, c * 128:(c + 1) * 128]
            )

        # ---- encoder: gather + mm1 + relu + mean-pool ----
        do_enc = parts in ("all", "enc", "enc-nocc")
        do_cc = parts in ("all", "enc")
        do_trans = parts in ("all", "trans")
        pooled_ps = smps.tile([BP, H], f32, tag="small")
        for g in range(GCHUNKS if do_enc else 0):
            xg = gpool.tile([128, 4, GCH], f16)
            nc.gpsimd.dma_gather(
                out_ap=xg[:],
                in_ap=table[:],
                idxs_ap=idx_sb[:, g * (GCH // 16):(g + 1) * (GCH // 16)],
                num_idxs=GCH,
                num_idxs_reg=GCH,
                elem_size=D,
                transpose=True,
                single_packet=False,
            )
            for it in range(CTILES):
                t = g * CTILES + it
                h_ps = mmps.tile([128, H], f32, tag="mm")
                # bias broadcast row (K=1 matmul), then 4 K-tiles of X @ We1
                nc.tensor.matmul(
                    h_ps[:], lhsT=ones1_sb[:1, :], rhs=be1_sb[:1, :],
                    start=True, stop=False,
                )
                for k in range(4):
                    nc.tensor.matmul(
                        h_ps[:],
                        lhsT=xg[:, k, it * 128:(it + 1) * 128],
                        rhs=we1_sb[:, k, :],
                        start=False, stop=(k == 3),
                    )
                h_sb = hpool.tile([128, H], f16)
                nc.scalar.activation(h_sb[:], h_ps[:], AF.Relu)
                nc.tensor.matmul(
                    pooled_ps[:],
                    lhsT=mask_sb[:, t * BP:(t + 1) * BP],
                    rhs=h_sb[:],
                    start=(t == 0), stop=(t == ITILES - 1),
                )

        if not do_enc:
            nc.vector.memset(pooled_ps[:], 0.0)
        pooled_sb = singles.tile([BP, H], f32)
        nc.vector.tensor_scalar_mul(pooled_sb[:], pooled_ps[:], 1.0 / S)
        pooledT_sb = singles.tile([128, 4, BP], f16)
        for c in range(4):
            pT_ps = tpps.tile([128, BP], f32, tag="tp")
            nc.tensor.transpose(
                out=pT_ps[:],
                in_=pooled_sb[:, c * 128:(c + 1) * 128],
                identity=ident_sb[:BP, :BP],
            )
            nc.vector.tensor_copy(pooledT_sb[:, c, :], pT_ps[:])

        enc_ps = smps.tile([BP, E], f32, tag="small")
        nc.tensor.matmul(
            enc_ps[:], lhsT=ones1_sb[:1, :BP], rhs=be2_sb[:1, :],
            start=True, stop=False,
        )
        for c in range(4):
            nc.tensor.matmul(
                enc_ps[:], lhsT=pooledT_sb[:, c, :], rhs=we2_sb[:, c, :],
                start=False, stop=(c == 3),
            )
        enc_sb = singles.tile([BP, E], f32)
        nc.vector.tensor_copy(enc_sb[:], enc_ps[:])
        nc.sync.dma_start(out=enco[:], in_=enc_sb[:])

        # ---- AllGather encodings across the 8 cores ----
        cc_in = dram.tile([BP, E], f32)
        cc_out = dram.tile([B, E], f32, addr_space="Shared")
        nc.sync.dma_start(out=cc_in[:], in_=enc_sb[:])
        if do_cc:
            nc.gpsimd.collective_compute(
                "AllGather",
                ALU.bypass,
                replica_groups=[list(range(NCORES))],
                ins=[cc_in[:]],
                outs=[cc_out[:]],
            )
        else:
            nc.sync.dma_start(out=cc_out[:BP, :], in_=enc_sb[:])

        # l2-normalize all B encodings, then transpose to [E, B] fp16
        encT_sb = singles.tile([128, 2, B], f16)
        encN_sb = singles.tile([128, 2, E], f32)
        for i in range(2):  # batch halves
            encf = spool.tile([128, E], f32, tag="encf")
            nc.sync.dma_start(out=encf[:], in_=cc_out[i * 128:(i + 1) * 128, :])
            sq = spool.tile([128, E], f32, tag="encsq")
            nc.vector.tensor_mul(sq[:], encf[:], encf[:])
            ss = spool.tile([128, 1], f32, tag="encss")
            nc.vector.tensor_reduce(
                ss[:], sq[:], axis=mybir.AxisListType.X, op=ALU.add
            )
            nc.vector.tensor_scalar_max(ss[:], ss[:], EPS)
            sroot = spool.tile([128, 1], f32, tag="encsr")
            nc.scalar.activation(sroot[:], ss[:], AF.Sqrt)
            rb = spool.tile([128, 1], f32, tag="encrb")
            nc.vector.reciprocal(rb[:], sroot[:])
            nc.vector.tensor_scalar_mul(encN_sb[:, i, :], encf[:], rb[:, :1])
        for i in range(2):      # batch halves
            for j in range(2):  # embed halves
                eT_ps = tpps.tile([128, 128], f32, tag="tp")
                nc.tensor.transpose(
                    out=eT_ps[:],
                    in_=encN_sb[:, i, j * 128:(j + 1) * 128],
                    identity=ident_sb[:],
                )
                nc.vector.tensor_copy(
                    encT_sb[:, j, i * 128:(i + 1) * 128], eT_ps[:]
                )

        # ---- transform: th = relu(shard @ Wt1 + bt1); t2 = th @ Wt2 + bt2 ----
        th_sb = singles.tile([128, 4, SHARD], f16)
        for h in range(4 if do_trans else 0):
            for n in range(NTT):
                th_ps = mmps.tile([128, NT], f32, tag="mm")
                for k in range(4):
                    nc.tensor.matmul(
                        th_ps[:],
                        lhsT=wt1_sb[:, k, h * 128:(h + 1) * 128],
                        rhs=shardT_sb[:, k, n * NT:(n + 1) * NT],
                        start=(k == 0), stop=(k == 3),
                    )
                nc.scalar.activation(
                    th_sb[:, h, n * NT:(n + 1) * NT], th_ps[:],
                    AF.Relu, bias=bt1_sb[:, h:h + 1],
                )
        t2_sb = singles.tile([128, 2, SHARD], f16)
        if not do_trans:
            nc.vector.memset(t2_sb[:], 0.1)
        for e in range(2 if do_trans else 0):
            for n in range(NTT):
                t2_ps = mmps.tile([128, NT], f32, tag="mm")
                for k in range(4):
                    nc.tensor.matmul(
                        t2_ps[:],
                        lhsT=wt2_sb[:, k, e * 128:(e + 1) * 128],
                        rhs=th_sb[:, k, n * NT:(n + 1) * NT],
                        start=(k == 0), stop=(k == 3),
                    )
                nc.scalar.activation(
                    t2_sb[:, e, n * NT:(n + 1) * NT], t2_ps[:],
                    AF.Identity, bias=bt2_sb[:, e:e + 1],
                )

        # ---- per-card sum of squares -> rsqrt scale ----
        ssq_ps = sqps.tile([128, CARD_TILES], f32, tag="ssq")
        for t in range(CARD_TILES):
            for e in range(2):
                sqt = spool.tile([128, 128], f16, tag="sqt")
                nc.vector.tensor_mul(
                    sqt[:],
                    t2_sb[:, e, t * 128:(t + 1) * 128],
                    t2_sb[:, e, t * 128:(t + 1) * 128],
                )
                nc.tensor.matmul(
                    ssq_ps[:, t:t + 1], lhsT=sqt[:], rhs=onesE_sb[:, :1],
                    start=(e == 0), stop=(e == 1),
                )
        ssq_sb = singles.tile([128, CARD_TILES], f32)
        nc.vector.tensor_scalar_max(ssq_sb[:], ssq_ps[:], EPS)
        sroot2 = singles.tile([128, CARD_TILES], f32)
        nc.scalar.activation(sroot2[:], ssq_sb[:], AF.Sqrt)
        rinv = singles.tile([128, CARD_TILES], f32)
        nc.vector.reciprocal(rinv[:], sroot2[:])
        rT = singles.tile([128, CARD_TILES], f32)
        nc.vector.tensor_mul(
            rT[:], rinv[:], temp_sb[:, :1].to_broadcast([128, CARD_TILES])
        )
        negb = singles.tile([128, 1], f32)
        nc.vector.tensor_scalar_mul(negb[:], temp_sb[:], -0.5)

        # ---- similarities + sigmoid ----
        for t in range(CARD_TILES):
            sims_ps = mmps.tile([128, B], f32, tag="mm")
            for e in range(2):
                nc.tensor.matmul(
                    sims_ps[:],
                    lhsT=t2_sb[:, e, t * 128:(t + 1) * 128],
                    rhs=encT_sb[:, e, :],
                    start=(e == 0), stop=(e == 1),
                )
            dec_sb = outp.tile([128, B], f32)
            nc.scalar.activation(
                dec_sb[:], sims_ps[:], AF.Sigmoid,
                bias=negb[:, :1], scale=rT[:, t:t + 1],
            )
            nc.sync.dma_start(out=decT[t * 128:(t + 1) * 128, :], in_=dec_sb[:])

    nc.finalize()
    return nc


def make_in_maps(inputs):
    noisy = np.asarray(inputs["noisy_cube"])
    cards = np.asarray(inputs["card_embeddings"], dtype=np.float32)
    W_e1 = np.asarray(inputs["W_e1"], dtype=np.float32)
    b_e1 = np.asarray(inputs["b_e1"], dtype=np.float32)
    W_e2 = np.asarray(inputs["W_e2"], dtype=np.float32)
    b_e2 = np.asarray(inputs["b_e2"], dtype=np.float32)
    W_t1 = np.asarray(inputs["W_t1"], dtype=np.float32)
    b_t1 = np.asarray(inputs["b_t1"], dtype=np.float32)
    W_t2 = np.asarray(inputs["W_t2"], dtype=np.float32)
    b_t2 = np.asarray(inputs["b_t2"], dtype=np.float32)
    temperature = float(np.asarray(inputs["temperature"]))

    table16 = np.zeros((NPAD, D), np.float16)
    table16[:NUM_CARDS] = cards.astype(np.float16)

    def tile_w(w):  # [K, N] -> [128, K//128, N]
        K, Nn = w.shape
        return np.ascontiguousarray(
            w.astype(np.float16).reshape(K // 128, 128, Nn).transpose(1, 0, 2)
        )

    we1 = tile_w(W_e1)
    we2 = tile_w(W_e2)
    wt1 = tile_w(W_t1)
    wt2 = tile_w(W_t2)
    bt1 = np.ascontiguousarray(b_t1.reshape(4, 128).T).astype(np.float32)
    bt2 = np.ascontiguousarray(b_t2.reshape(2, 128).T).astype(np.float32)

    mask = np.zeros((128, ITILES * BP), np.float16)
    for t in range(ITILES):
        items = t * 128 + np.arange(128)
        b = items // S
        mask[np.arange(128), t * BP + b] = 1.0

    in_maps = []
    for m in range(NCORES):
        idx_m = noisy[m * BP:(m + 1) * BP].reshape(-1).astype(np.int16)
        idx_wrap = np.ascontiguousarray(np.tile(idx_m.reshape(ITEMS // 16, 16).T, (8, 1)))
        in_maps.append({
            "table": table16,
            "shard": np.ascontiguousarray(table16[m * SHARD:(m + 1) * SHARD]),
            "idx": idx_wrap,
            "maskd": mask,
            "we1": we1, "we2": we2, "wt1": wt1, "wt2": wt2,
            "be1": np.ascontiguousarray(b_e1.astype(np.float16)[None, :]),
            "be2": np.ascontiguousarray(b_e2.astype(np.float16)[None, :]),
            "bt1": bt1, "bt2": bt2,
            "temp": np.full((128, 1), temperature, np.float32),
        })
    return in_maps


def assemble_outputs(results):
    decT = np.concatenate([results[m]["decT"] for m in range(NCORES)], axis=0)
    decoded = np.ascontiguousarray(decT[:NUM_CARDS].T[:, 1:]).astype(np.float32)
    encoded = np.concatenate(
        [results[m]["enco"] for m in range(NCORES)], axis=0
    ).astype(np.float32)
    return decoded, encoded


def kernel(**inputs):
    from concourse.bass_utils import run_bass_kernel_spmd

    if "nc" not in _CACHE:
        _CACHE["nc"] = build_program()
    nc = _CACHE["nc"]
    in_maps = make_in_maps(inputs)
    res = run_bass_kernel_spmd(nc, in_maps, list(range(NCORES))).results
    return assemble_outputs(res)


# revision 14
# speedup vs baseline: 1.4195x; 1.4195x over previous
"""CubeRecommender Trainium2 kernel (8-core SPMD).

Strategy:
  - Gallery (card) axis sharded 8 ways for the TransformCards MLP + similarity.
  - Batch axis sharded 8 ways for the EmbedCube set-encoder (gather + MLP +
    mean-pool); the tiny [B,E] cube encodings are AllGathered on-device.
  - Matmul inputs in fp16 (fp32 PSUM accumulation); l2norm / sigmoid in fp32.

Device program (identical on all 8 cores, per-core data differs):
  encoder:  dma_gather(transpose=True) pulls X^T tiles [128_D, 4, items] from
            the fp16 gallery (each batch padded 360->368 slots, pads point at
            a zero gallery row); h^T = relu(We1^T X^T + be1) with We1 tiles
            stationary; the mean-pool is free via the activation's accum_out
            row-sum (pad contribution 8*relu(be1) subtracted exactly);
            enc = pooled @ We2 + be2; AllGather enc across cores.
  transform: shard^T via xbar DMA-transpose; th^T = relu(Wt1^T shard^T + bt1);
            t2^T = Wt2^T th^T + bt2  ([E, cards], cards on the free axis).
  sims:     per-card sumsq via squared-tile x ones matmul; decoded^T tile =
            sigmoid(raw_sims * (rsqrt(sumsq)*T) - 0.5*T) with per-partition
            activation scale/bias (card normalization folded into the scale).

Host glue: shard/layout prep, fp16 cast, and final unshard/transpose only.
"""

import numpy as np

NUM_CARDS = 30000
NPAD = 30720          # 8 * 3840, zero-padded gallery (row 30000+ is zeros)
SHARD = 3840          # cards per core (uniform SPMD shapes)
D = 512
H = 512
E = 256
B = 256
BP = 32               # batches per core
S = 360
SPB = 368             # padded slots per batch (8 pad idxs -> zero row)
PAD = SPB - S
NCORES = 8
ITEMS = BP * SPB      # 11776 gathered rows per core
GCHUNKS = 4
GCH = ITEMS // GCHUNKS   # 2944 indices per gather (= 8 batches, %128 == 0)
WPC = GCH // SPB         # 8 windows (batches) per chunk
WG = 4                   # windows per PSUM group
NT = 480               # transform free-dim tile (3840 = 8*480)
NTT = SHARD // NT      # 8
CARD_TILES = SHARD // 128  # 30
EPS = 1e-12

_CACHE = {}


def build_program():
    import concourse.mybir as mybir
    import concourse.tile as tile
    from contextlib import ExitStack
    from concourse.masks import make_identity
    from concourse import bacc, library_config

    f16 = mybir.dt.float16
    f32 = mybir.dt.float32
    i16 = mybir.dt.int16
    AF = mybir.ActivationFunctionType
    ALU = mybir.AluOpType

    nc = bacc.Bacc("TRN2", target_bir_lowering=False, num_devices=NCORES)

    table = nc.dram_tensor("table", [NPAD, D], f16, kind="ExternalInput")
    shard = nc.dram_tensor("shard", [SHARD, D], f16, kind="ExternalInput")
    idx = nc.dram_tensor("idx", [128, ITEMS // 16], i16, kind="ExternalInput")
    we1 = nc.dram_tensor("we1", [128, 4, H], f16, kind="ExternalInput")
    we2 = nc.dram_tensor("we2", [128, 4, E], f16, kind="ExternalInput")
    wt1 = nc.dram_tensor("wt1", [128, 4, H], f16, kind="ExternalInput")
    wt2 = nc.dram_tensor("wt2", [128, 4, E], f16, kind="ExternalInput")
    be1 = nc.dram_tensor("be1", [128, 4], f32, kind="ExternalInput")
    be2 = nc.dram_tensor("be2", [1, E], f16, kind="ExternalInput")
    bt1 = nc.dram_tensor("bt1", [128, 4], f32, kind="ExternalInput")
    bt2 = nc.dram_tensor("bt2", [128, 2], f32, kind="ExternalInput")
    temp = nc.dram_tensor("temp", [128, 1], f32, kind="ExternalInput")

    decT = nc.dram_tensor("decT", [SHARD, B], f32, kind="ExternalOutput")
    enco = nc.dram_tensor("enco", [BP, E], f32, kind="ExternalOutput")

    with ExitStack() as ctx:
        tc = ctx.enter_context(tile.TileContext(nc))
        singles = ctx.enter_context(tc.tile_pool(name="singles", bufs=1))
        dram = ctx.enter_context(tc.tile_pool(name="dram", bufs=1, space="DRAM"))
        gpool = ctx.enter_context(tc.tile_pool(name="gpool", bufs=2))
        hpool = ctx.enter_context(tc.tile_pool(name="hpool", bufs=3))
        spool = ctx.enter_context(tc.tile_pool(name="spool", bufs=2))
        outp = ctx.enter_context(tc.tile_pool(name="outp", bufs=3))
        mmps = ctx.enter_context(tc.tile_pool(name="mmps", bufs=4, space="PSUM"))
        smps = ctx.enter_context(tc.tile_pool(name="smps", bufs=1, space="PSUM"))
        sqps = ctx.enter_context(tc.tile_pool(name="sqps", bufs=1, space="PSUM"))
        tpps = ctx.enter_context(tc.tile_pool(name="tpps", bufs=1, space="PSUM"))

        # ---- constants / weights into SBUF ----
        we1_sb = singles.tile([128, 4, H], f16)
        nc.sync.dma_start(out=we1_sb[:], in_=we1[:])
        we2_sb = singles.tile([128, 4, E], f16)
        nc.sync.dma_start(out=we2_sb[:], in_=we2[:])
        wt1_sb = singles.tile([128, 4, H], f16)
        nc.sync.dma_start(out=wt1_sb[:], in_=wt1[:])
        wt2_sb = singles.tile([128, 4, E], f16)
        nc.sync.dma_start(out=wt2_sb[:], in_=wt2[:])
        be1_sb = singles.tile([128, 4], f32)
        nc.sync.dma_start(out=be1_sb[:], in_=be1[:])
        be2_sb = singles.tile([1, E], f16)
        nc.sync.dma_start(out=be2_sb[:], in_=be2[:])
        bt1_sb = singles.tile([128, 4], f32)
        nc.sync.dma_start(out=bt1_sb[:], in_=bt1[:])
        bt2_sb = singles.tile([128, 2], f32)
        nc.sync.dma_start(out=bt2_sb[:], in_=bt2[:])
        temp_sb = singles.tile([128, 1], f32)
        nc.sync.dma_start(out=temp_sb[:], in_=temp[:])
        idx_sb = singles.tile([128, ITEMS // 16], i16)
        nc.sync.dma_start(out=idx_sb[:], in_=idx[:])

        ones1_sb = singles.tile([1, BP], f16)
        nc.vector.memset(ones1_sb[:], 1.0)
        onesE_sb = singles.tile([128, 1], f16)
        nc.vector.memset(onesE_sb[:], 1.0)
        ident_sb = singles.tile([128, 128], f32)
        make_identity(nc, ident_sb[:])
        nc.gpsimd.load_library(library_config.mlp)

        # shard^T: [128_D, 4, SHARD] fp16 via xbar DMA-transpose
        shardT_sb = singles.tile([128, 4, SHARD], f16)
        for c in range(4):
            nc.sync.dma_start_transpose(
                out=shardT_sb[:, c, :], in_=shard[:, c * 128:(c + 1) * 128]
            )

        # ---- encoder: gather + weight-stationary mm1 + relu/accum pooling ----
        pooledT_sb = singles.tile([128, 4, BP], f32)
        for g in range(GCHUNKS):
            xg = gpool.tile([128, 4, GCH], f16)
            nc.gpsimd.dma_gather(
                out_ap=xg[:],
                in_ap=table[:],
                idxs_ap=idx_sb[:, g * (GCH // 16):(g + 1) * (GCH // 16)],
                num_idxs=GCH,
                num_idxs_reg=GCH,
                elem_size=D,
                transpose=True,
                single_packet=False,
            )
            for h in range(4):
                for wg in range(WPC // WG):
                    hps = [mmps.tile([128, SPB], f32, tag="mm", name=f"hps{i}")
                           for i in range(WG)]
                    for k in range(4):
                        for i in range(WG):
                            w = wg * WG + i
                            nc.tensor.matmul(
                                hps[i][:],
                                lhsT=we1_sb[:, k, h * 128:(h + 1) * 128],
                                rhs=xg[:, k, w * SPB:(w + 1) * SPB],
                                start=(k == 0), stop=(k == 3),
                            )
                    for i in range(WG):
                        w = wg * WG + i
                        b_glob = g * WPC + w
                        hscr = hpool.tile([128, SPB], f16)
                        nc.scalar.activation(
                            hscr[:], hps[i][:], AF.Relu,
                            bias=be1_sb[:, h:h + 1],
                            accum_out=pooledT_sb[:, h, b_glob:b_glob + 1],
                        )

        # pad slots gathered the zero row: remove their PAD*relu(be1) term,
        # apply the 1/S mean, cast to fp16 for mm2.
        rb8 = singles.tile([128, 4], f32)
        nc.scalar.activation(rb8[:], be1_sb[:], AF.Relu, scale=float(PAD))
        for h in range(4):
            nc.vector.tensor_scalar_sub(
                pooledT_sb[:, h, :], pooledT_sb[:, h, :], rb8[:, h:h + 1]
            )
        pooledT16 = singles.tile([128, 4, BP], f16)
        nc.vector.tensor_scalar_mul(pooledT16[:], pooledT_sb[:], 1.0 / S)

        enc_ps = smps.tile([BP, E], f32, tag="small")
        nc.tensor.matmul(
            enc_ps[:], lhsT=ones1_sb[:1, :], rhs=be2_sb[:1, :],
            start=True, stop=False,
        )
        for c in range(4):
            nc.tensor.matmul(
                enc_ps[:], lhsT=pooledT16[:, c, :], rhs=we2_sb[:, c, :],
                start=False, stop=(c == 3),
            )
        enc_sb = singles.tile([BP, E], f32)
        nc.vector.tensor_copy(enc_sb[:], enc_ps[:])
        nc.sync.dma_start(out=enco[:], in_=enc_sb[:])

        # ---- AllGather encodings across the 8 cores ----
        cc_in = dram.tile([BP, E], f32)
        cc_out = dram.tile([B, E], f32, addr_space="Shared")
        nc.sync.dma_start(out=cc_in[:], in_=enc_sb[:])
        nc.gpsimd.collective_compute(
            "AllGather",
            ALU.bypass,
            replica_groups=[list(range(NCORES))],
            ins=[cc_in[:]],
            outs=[cc_out[:]],
        )

        # l2-normalize all B encodings, then transpose to [E, B] fp16
        encT_sb = singles.tile([128, 2, B], f16)
        encN_sb = singles.tile([128, 2, E], f32)
        for i in range(2):  # batch halves
            encf = spool.tile([128, E], f32, tag="encf")
            nc.sync.dma_start(out=encf[:], in_=cc_out[i * 128:(i + 1) * 128, :])
            sq = spool.tile([128, E], f32, tag="encsq")
            nc.vector.tensor_mul(sq[:], encf[:], encf[:])
            ss = spool.tile([128, 1], f32, tag="encss")
            nc.vector.tensor_reduce(
                ss[:], sq[:], axis=mybir.AxisListType.X, op=ALU.add
            )
            nc.vector.tensor_scalar_max(ss[:], ss[:], EPS)
            sroot = spool.tile([128, 1], f32, tag="encsr")
            nc.scalar.activation(sroot[:], ss[:], AF.Sqrt)
            rb = spool.tile([128, 1], f32, tag="encrb")
            nc.vector.reciprocal(rb[:], sroot[:])
            nc.vector.tensor_scalar_mul(encN_sb[:, i, :], encf[:], rb[:, :1])
        for i in range(2):      # batch halves
            for j in range(2):  # embed halves
                eT_ps = tpps.tile([128, 128], f32, tag="tp")
                nc.tensor.transpose(
                    out=eT_ps[:],
                    in_=encN_sb[:, i, j * 128:(j + 1) * 128],
                    identity=ident_sb[:],
                )
                nc.vector.tensor_copy(
                    encT_sb[:, j, i * 128:(i + 1) * 128], eT_ps[:]
                )

        # ---- transform: th^T = relu(Wt1^T shard^T + bt1); t2^T = Wt2^T th^T ----
        th_sb = singles.tile([128, 4, SHARD], f16)
        for h in range(4):
            for ng in range(NTT // WG):
                tps = [mmps.tile([128, NT], f32, tag="mm", name=f"tps{i}")
                       for i in range(WG)]
                for k in range(4):
                    for i in range(WG):
                        n = ng * WG + i
                        nc.tensor.matmul(
                            tps[i][:],
                            lhsT=wt1_sb[:, k, h * 128:(h + 1) * 128],
                            rhs=shardT_sb[:, k, n * NT:(n + 1) * NT],
                            start=(k == 0), stop=(k == 3),
                        )
                for i in range(WG):
                    n = ng * WG + i
                    nc.scalar.activation(
                        th_sb[:, h, n * NT:(n + 1) * NT], tps[i][:],
                        AF.Relu, bias=bt1_sb[:, h:h + 1],
                    )
        t2_sb = singles.tile([128, 2, SHARD], f16)
        for e in range(2):
            for ng in range(NTT // WG):
                tps = [mmps.tile([128, NT], f32, tag="mm", name=f"tps{i}")
                       for i in range(WG)]
                for k in range(4):
                    for i in range(WG):
                        n = ng * WG + i
                        nc.tensor.matmul(
                            tps[i][:],
                            lhsT=wt2_sb[:, k, e * 128:(e + 1) * 128],
                            rhs=th_sb[:, k, n * NT:(n + 1) * NT],
                            start=(k == 0), stop=(k == 3),
                        )
                for i in range(WG):
                    n = ng * WG + i
                    nc.scalar.activation(
                        t2_sb[:, e, n * NT:(n + 1) * NT], tps[i][:],
                        AF.Identity, bias=bt2_sb[:, e:e + 1],
                    )

        # ---- per-card sum of squares -> rsqrt scale ----
        ssq_ps = sqps.tile([128, CARD_TILES], f32, tag="ssq")
        for t in range(CARD_TILES):
            for e in range(2):
                sqt = spool.tile([128, 128], f16, tag="sqt")
                nc.vector.tensor_mul(
                    sqt[:],
                    t2_sb[:, e, t * 128:(t + 1) * 128],
                    t2_sb[:, e, t * 128:(t + 1) * 128],
                )
                nc.tensor.matmul(
                    ssq_ps[:, t:t + 1], lhsT=sqt[:], rhs=onesE_sb[:, :1],
                    start=(e == 0), stop=(e == 1),
                )
        ssq_sb = singles.tile([128, CARD_TILES], f32)
        nc.vector.tensor_scalar_max(ssq_sb[:], ssq_ps[:], EPS)
        sroot2 = singles.tile([128, CARD_TILES], f32)
        nc.scalar.activation(sroot2[:], ssq_sb[:], AF.Sqrt)
        rinv = singles.tile([128, CARD_TILES], f32)
        nc.vector.reciprocal(rinv[:], sroot2[:])
        rT = singles.tile([128, CARD_TILES], f32)
        nc.vector.tensor_mul(
            rT[:], rinv[:], temp_sb[:, :1].to_broadcast([128, CARD_TILES])
        )
        negb = singles.tile([128, 1], f32)
        nc.vector.tensor_scalar_mul(negb[:], temp_sb[:], -0.5)

        # ---- similarities + sigmoid ----
        for t in range(CARD_TILES):
            sims_ps = mmps.tile([128, B], f32, tag="mm")
            for e in range(2):
                nc.tensor.matmul(
                    sims_ps[:],
                    lhsT=t2_sb[:, e, t * 128:(t + 1) * 128],
                    rhs=encT_sb[:, e, :],
                    start=(e == 0), stop=(e == 1),
                )
            dec_sb = outp.tile([128, B], f32)
            nc.scalar.activation(
                dec_sb[:], sims_ps[:], AF.Sigmoid,
                bias=negb[:, :1], scale=rT[:, t:t + 1],
            )
            nc.sync.dma_start(out=decT[t * 128:(t + 1) * 128, :], in_=dec_sb[:])

    nc.finalize()
    return nc


def make_in_maps(inputs):
    noisy = np.asarray(inputs["noisy_cube"])
    cards = np.asarray(inputs["card_embeddings"], dtype=np.float32)
    W_e1 = np.asarray(inputs["W_e1"], dtype=np.float32)
    b_e1 = np.asarray(inputs["b_e1"], dtype=np.float32)
    W_e2 = np.asarray(inputs["W_e2"], dtype=np.float32)
    b_e2 = np.asarray(inputs["b_e2"], dtype=np.float32)
    W_t1 = np.asarray(inputs["W_t1"], dtype=np.float32)
    b_t1 = np.asarray(inputs["b_t1"], dtype=np.float32)
    W_t2 = np.asarray(inputs["W_t2"], dtype=np.float32)
    b_t2 = np.asarray(inputs["b_t2"], dtype=np.float32)
    temperature = float(np.asarray(inputs["temperature"]))

    table16 = np.zeros((NPAD, D), np.float16)
    table16[:NUM_CARDS] = cards.astype(np.float16)

    def tile_w(w):  # [K, N] -> [128, K//128, N]
        K, Nn = w.shape
        return np.ascontiguousarray(
            w.astype(np.float16).reshape(K // 128, 128, Nn).transpose(1, 0, 2)
        )

    we1 = tile_w(W_e1)
    we2 = tile_w(W_e2)
    wt1 = tile_w(W_t1)
    wt2 = tile_w(W_t2)
    be1 = np.ascontiguousarray(b_e1.reshape(4, 128).T).astype(np.float32)
    bt1 = np.ascontiguousarray(b_t1.reshape(4, 128).T).astype(np.float32)
    bt2 = np.ascontiguousarray(b_t2.reshape(2, 128).T).astype(np.float32)

    in_maps = []
    for m in range(NCORES):
        idx_pad = np.full((BP, SPB), NUM_CARDS, np.int16)
        idx_pad[:, :S] = noisy[m * BP:(m + 1) * BP].astype(np.int16)
        idx_m = idx_pad.reshape(-1)
        idx_wrap = np.ascontiguousarray(
            np.tile(idx_m.reshape(ITEMS // 16, 16).T, (8, 1))
        )
        in_maps.append({
            "table": table16,
            "shard": np.ascontiguousarray(table16[m * SHARD:(m + 1) * SHARD]),
            "idx": idx_wrap,
            "we1": we1, "we2": we2, "wt1": wt1, "wt2": wt2,
            "be1": be1,
            "be2": np.ascontiguousarray(b_e2.astype(np.float16)[None, :]),
            "bt1": bt1, "bt2": bt2,
            "temp": np.full((128, 1), temperature, np.float32),
        })
    return in_maps


def assemble_outputs(results):
    decT = np.concatenate([results[m]["decT"] for m in range(NCORES)], axis=0)
    decoded = np.ascontiguousarray(decT[:NUM_CARDS].T[:, 1:]).astype(np.float32)
    encoded = np.concatenate(
        [results[m]["enco"] for m in range(NCORES)], axis=0
    ).astype(np.float32)
    return decoded, encoded


def kernel(**inputs):
    from concourse.bass_utils import run_bass_kernel_spmd

    if "nc" not in _CACHE:
        _CACHE["nc"] = build_program()
    nc = _CACHE["nc"]
    in_maps = make_in_maps(inputs)
    res = run_bass_kernel_spmd(nc, in_maps, list(range(NCORES))).results
    return assemble_outputs(res)
